# revision 1
# baseline (speedup 1.0000x reference)
"""Trainium2 Bass kernel for ContrastiveResNetGCN.

Reference computation (N=8192, D=512, P=128, H=128):
    x_proj = relu(x1 @ W1) @ W2                       [N,P]
    A      = cos_sim(x_proj)  (eps clamp never binds) [N,N]
    out    = (A @ (x1 @ gc_w)) / N + gc_b             [N,H]

Since the eps clamp never binds (row norms ~5-10 >> 1e-8),
    A = g @ g.T  with g = x_proj / ||x_proj||_row
so  A @ h = g @ (g.T @ h) — a rank-P factorization that removes both
[N,N] matmuls.  Per core (rows sharded 8 x 1024):
    u   = x_proj (unnormalized)      [R,P]
    w   = row_norms(u); v = h * (1/w) * (1/N)
    Mp  = u.T @ v                    [P,H]   (partial, AllGather+sum)
    out = (u @ M) * (1/w) + gc_b

On-chip layouts: [feat, node] ("T layout", weights act as pre-transposed
lhsT) for the projector chain, [node, feat] ("N layout") for the row-norm
scaling and the M contraction over nodes.

`nreps` unrolls the whole computation nreps times inside one NEFF (tiles
reused => dependency-serialized), which amortizes the axon dispatch
overhead for timing: exec_ns = slope of wall(nreps).
"""

import os
import sys

import numpy as np

for _p in ("/opt/trn_rl_repo", "/opt/pypackages"):
    if os.path.isdir(_p) and _p not in sys.path:
        sys.path.append(_p)

import concourse.bass as bass
import concourse.mybir as mybir
from concourse import bacc
import concourse.tile as tile
from concourse.bass_utils import run_bass_kernel_spmd
from concourse.masks import make_identity

F32 = mybir.dt.float32
F32R = mybir.dt.float32r
AF = mybir.ActivationFunctionType
ALU = mybir.AluOpType

N, D, P, H = 8192, 512, 128, 128
NCORES = 8
R = N // NCORES          # rows per core
NT = R // 128            # node tiles per core (8)
KD = D // 128            # contraction tiles over D (4)
NG = R // 512            # node groups of 512 (2)
INV_N = 1.0 / N


def sl(i, w=128):
    return slice(i * w, (i + 1) * w)


def build_bass(nreps: int = 1, no_cc: bool = False):
    nc = bacc.Bacc("TRN2", target_bir_lowering=False, num_devices=NCORES)

    x1 = nc.dram_tensor("x1", [R, D], F32, kind="ExternalInput")
    w1 = nc.dram_tensor("w1", [D, D], F32, kind="ExternalInput")
    w2 = nc.dram_tensor("w2", [D, P], F32, kind="ExternalInput")
    gcw = nc.dram_tensor("gcw", [D, H], F32, kind="ExternalInput")
    gcb = nc.dram_tensor("gcb", [H], F32, kind="ExternalInput")
    out = nc.dram_tensor("out", [R, H], F32, kind="ExternalOutput")

    with tile.TileContext(nc) as tc:
        with (
            tc.tile_pool(name="cpool", bufs=1) as cpool,
            tc.tile_pool(name="xload", bufs=8) as xload,
            tc.tile_pool(name="scratch", bufs=2) as spool,
            tc.tile_pool(name="opool", bufs=3) as opool,
            tc.tile_pool(name="pbig", bufs=3, space="PSUM") as pbig,
            tc.tile_pool(name="psmall", bufs=4, space="PSUM") as psmall,
            tc.tile_pool(name="pmm", bufs=1, space="PSUM") as pmm,
            # psmall is the shared [128,512]-slot pool (tag "ps") used for
            # batched transposes and the tail matmuls
            tc.tile_pool(name="dram", bufs=1, space="DRAM") as dram,
        ):
            # ---- constants / weights (loaded once) ---------------------
            ident = cpool.tile([128, 128], F32, name="ident")
            make_identity(nc, ident)

            # x1 row tiles for rep 0 issue FIRST: they head the critical
            # path (transposes) and must not queue behind 1.3MB of weights
            # on the HWDGE FIFO.
            w1t, w2t, gcwt = [], [], []
            for kd in range(KD):
                t = cpool.tile([128, D], F32, name=f"w1_{kd}")
                nc.sync.dma_start(out=t, in_=w1[sl(kd), :])
                w1t.append(t)
            for kd in range(KD):
                t = cpool.tile([128, P], F32, name=f"w2_{kd}")
                nc.sync.dma_start(out=t, in_=w2[sl(kd), :])
                w2t.append(t)
            for kd in range(KD):
                t = cpool.tile([128, H], F32, name=f"gcw_{kd}")
                nc.sync.dma_start(out=t, in_=gcw[sl(kd), :])
                gcwt.append(t)

            # f32r-rounded weight copies (one-time): fp32r matmul operands
            # must be produced rounded-to-f32r by their writer.
            w1r, w2r, gcwr = [], [], []
            for kd in range(KD):
                t = cpool.tile([128, D], F32R, name=f"w1r_{kd}")
                nc.vector.tensor_copy(t, w1t[kd])
                w1r.append(t)
            for kd in range(KD):
                t = cpool.tile([128, P], F32R, name=f"w2r_{kd}")
                nc.vector.tensor_copy(t, w2t[kd])
                w2r.append(t)
            for kd in range(KD):
                t = cpool.tile([128, H], F32R, name=f"gcwr_{kd}")
                nc.scalar.copy(t, gcwt[kd])
                gcwr.append(t)

            b_row = cpool.tile([1, H], F32, name="b_row")
            nc.sync.dma_start(out=b_row, in_=gcb[None, :])
            ones1 = cpool.tile([1, 128], F32, name="ones1")
            nc.vector.memset(ones1, 1.0)
            # bb[p, q] = gcb[q] for all p (partition-broadcast via K=1 matmul)
            bb = cpool.tile([128, H], F32, name="bb")
            pbb = psmall.tile([128, H], F32, name="pbb", tag="ps")
            nc.tensor.matmul(pbb, lhsT=ones1, rhs=b_row, start=True, stop=True)
            nc.scalar.copy(bb, pbb)

            # ---- persistent per-rep tiles (reused across reps) ---------
            x1T = [cpool.tile([128, R], F32R, name=f"x1T_{kd}") for kd in range(KD)]
            Bt = [cpool.tile([128, R], F32R, name=f"Bt_{mf}") for mf in range(KD)]
            ut = cpool.tile([128, R], F32, name="ut")
            hT = cpool.tile([128, R], F32, name="hT")
            u4 = [cpool.tile([128, 512], F32, name=f"u4_{g}") for g in range(NG)]
            v4 = [cpool.tile([128, 512], F32, name=f"v4_{g}") for g in range(NG)]
            winv = [cpool.tile([128, 1], F32, name=f"winv_{m}") for m in range(NT)]
            ssqs = [cpool.tile([128, 1], F32, name=f"ssq_{m}") for m in range(NT)]
            Mg = cpool.tile([128, NCORES * H], F32, name="Mg")
            Msb = cpool.tile([128, H], F32, name="Msb")

            persist = (ident, w1r, w2r, gcwr, bb,
                       x1T, Bt, ut, hT, u4, v4, winv, ssqs, Mg, Msb)
            pools = (xload, spool, opool, pbig, psmall, pmm, dram)
            for _rep in range(nreps):
                _emit_body(nc, x1, out, pools, persist, no_cc)

    nc.compile()
    return nc


def _emit_body(nc, x1, out, pools, persist, no_cc=False, xr_pre=None):
    (xload, spool, opool, pbig, psmall, pmm, dram) = pools
    (ident, w1t, w2t, gcwt, bb,
     x1T, Bt, ut, hT, u4, v4, winv, ssqs, Mg, Msb) = persist

    pM = pmm.tile([128, H], F32, name="pM", tag="pM")
    # ---- stage 1: all x1 rows loaded + PE-transposed into x1T ----------
    for g in range(NG):
        gs = sl(g, 512)
        xrs = []
        for j in range(4):
            xr = xload.tile([128, D], F32, name="xr")
            nc.sync.dma_start(out=xr, in_=x1[sl(4 * g + j), :])
            xrs.append(xr)
        for kd in range(KD):
            ptx = psmall.tile([128, 512], F32, name="ptx", tag="ps")
            for j in range(4):
                nc.tensor.transpose(ptx[:, sl(j)], xrs[j][:, sl(kd)], ident)
            if kd % 2 == 0:
                nc.vector.tensor_copy(x1T[kd][:, gs], ptx)
            else:
                nc.scalar.copy(x1T[kd][:, gs], ptx)

    # ---- stage 2+: chains, fully stage-interleaved across groups -------
    for g in range(NG):
        gs = sl(g, 512)
        for mf in range(KD):
            pb = pbig.tile([128, 512], F32, name="pb", tag="big")
            for kd in range(KD):
                nc.tensor.matmul(
                    pb,
                    lhsT=w1t[kd][:, sl(mf)],
                    rhs=x1T[kd][:, gs],
                    start=(kd == 0),
                    stop=(kd == KD - 1),
                )
            nc.scalar.activation(Bt[mf][:, gs], pb, AF.Relu)
    for g in range(NG):
        gs = sl(g, 512)
        pu = pbig.tile([128, 512], F32, name="pu", tag="big")
        for mf in range(KD):
            nc.tensor.matmul(
                pu,
                lhsT=w2t[mf],
                rhs=Bt[mf][:, gs],
                start=(mf == 0),
                stop=(mf == KD - 1),
            )
        nc.vector.tensor_copy(ut[:, gs], pu)
        ph = pbig.tile([128, 512], F32, name="ph", tag="big")
        for kd in range(KD):
            nc.tensor.matmul(
                ph,
                lhsT=gcwt[kd],
                rhs=x1T[kd][:, gs],
                start=(kd == 0),
                stop=(kd == KD - 1),
            )
        nc.scalar.copy(hT[:, gs], ph)

    # ---- back to node-major: u, norms, v, M partials --------------------
    for g in range(NG):
        ptu = psmall.tile([128, 512], F32, name="ptu", tag="ps")
        for j in range(4):
            nc.tensor.transpose(ptu[:, sl(j)], ut[:, sl(4 * g + j)], ident)
        nc.vector.tensor_copy(u4[g], ptu)
        for j in range(4):
            m = 4 * g + j
            sq = spool.tile([128, 128], F32, name="sq")
            nc.scalar.activation(sq, ptu[:, sl(j)], AF.Square,
                                 accum_out=ssqs[m])
            wv = spool.tile([128, 1], F32, name="wv")
            nc.scalar.activation(wv, ssqs[m], AF.Sqrt)
            nc.vector.reciprocal(winv[m], wv)

        pth = psmall.tile([128, 512], F32, name="pth", tag="ps")
        for j in range(4):
            nc.tensor.transpose(pth[:, sl(j)], hT[:, sl(4 * g + j)], ident)
        for j in range(4):
            m = 4 * g + j
            # v = h * winv * (1/N)  in one DVE op (fold inv_n: M needs no rescale)
            nc.vector.tensor_scalar(
                v4[g][:, sl(j)], pth[:, sl(j)], winv[m], INV_N,
                op0=ALU.mult, op1=ALU.mult,
            )
        for j in range(4):
            m = 4 * g + j
            nc.tensor.matmul(
                pM,
                lhsT=u4[g][:, sl(j)],
                rhs=v4[g][:, sl(j)],
                start=(m == 0),
                stop=(m == NT - 1),
            )

    # ---- AllGather partial M's and reduce locally ----------------------
    nc.scalar.copy(Msb, pM)
    if no_cc:  # timing-only variant: skip the collective (math wrong)
        Mred = Msb
        _finish_out(nc, out, opool, psmall, ut, winv, bb, Mred)
        return
    cc_in = dram.tile([128, H], F32, name="cc_in", tag="cc_in")
    cc_out = dram.tile([NCORES * 128, H], F32, name="cc_out",
                       addr_space="Shared", tag="cc_out")
    nc.sync.dma_start(out=cc_in, in_=Msb)
    nc.gpsimd.collective_compute(
        "AllGather",
        ALU.bypass,
        replica_groups=[list(range(NCORES))],
        ins=[cc_in[:, :]],
        outs=[cc_out[:, :]],
    )
    nc.sync.dma_start(
        out=Mg.rearrange("p (r f) -> p r f", r=NCORES),
        in_=cc_out.rearrange("(r p) f -> p r f", p=128),
    )
    nc.vector.tensor_add(Mg[:, 0:512], Mg[:, 0:512], Mg[:, 512:1024])
    nc.vector.tensor_add(Mg[:, 0:256], Mg[:, 0:256], Mg[:, 256:512])
    nc.vector.tensor_add(Mg[:, 0:128], Mg[:, 0:128], Mg[:, 128:256])
    Mred = Mg[:, 0:128]

    _finish_out(nc, out, opool, psmall, ut, winv, bb, Mred)


def _finish_out(nc, out, opool, psmall, ut, winv, bb, Mred):
    # ---- out = (u @ M) * winv + bb -------------------------------------
    for m in range(NT):
        pp = psmall.tile([128, H], F32, name="pp", tag="ps")
        nc.tensor.matmul(pp, lhsT=ut[:, sl(m)], rhs=Mred, start=True, stop=True)
        ob = opool.tile([128, H], F32, name="ob")
        nc.vector.scalar_tensor_tensor(
            ob, pp, winv[m], bb, op0=ALU.mult, op1=ALU.add
        )
        nc.sync.dma_start(out=out[sl(m), :], in_=ob)


_NCS = {}
LAST_RESULTS = None
_RUNNERS = {}


def _get_nc(nreps: int = 1):
    key = (nreps, os.environ.get("KERNEL_NO_CC") == "1")
    if key not in _NCS:
        _NCS[key] = build_bass(nreps, no_cc=key[1])
    return _NCS[key]


def _split_in_maps(inputs):
    x1 = np.ascontiguousarray(np.asarray(inputs["x1"], dtype=np.float32))
    w1 = np.ascontiguousarray(np.asarray(inputs["proj_w1"], dtype=np.float32))
    w2 = np.ascontiguousarray(np.asarray(inputs["proj_w2"], dtype=np.float32))
    gcw = np.ascontiguousarray(np.asarray(inputs["gc_w"], dtype=np.float32))
    gcb = np.ascontiguousarray(np.asarray(inputs["gc_b"], dtype=np.float32))
    return [
        {
            "x1": np.ascontiguousarray(x1[c * R:(c + 1) * R]),
            "w1": w1,
            "w2": w2,
            "gcw": gcw,
            "gcb": gcb,
        }
        for c in range(NCORES)
    ]


def kernel(**inputs) -> np.ndarray:
    global LAST_RESULTS
    res = run_bass_kernel_spmd(
        _get_nc(1), _split_in_maps(inputs), core_ids=list(range(NCORES))
    )
    LAST_RESULTS = res
    return np.concatenate([res.results[c]["out"] for c in range(NCORES)], axis=0)


# ---------------------------------------------------------------------------
# Timing path: the nreps-unrolled NEFF amortizes the (tens of ms) axon
# dispatch overhead; per-exec time = slope between two nreps points.
# ---------------------------------------------------------------------------

def _make_runner(nreps: int):
    if nreps in _RUNNERS:
        return _RUNNERS[nreps]
    import jax
    import concourse.mybir as mybir_
    from concourse.bass2jax import (
        _bass_exec_p,
        install_neuronx_cc_hook,
        partition_id_tensor,
    )
    from jax.experimental.shard_map import shard_map
    from jax.sharding import Mesh, PartitionSpec

    nc = _get_nc(nreps)
    install_neuronx_cc_hook()
    partition_name = (
        nc.partition_id_tensor.name if nc.partition_id_tensor else None
    )

    in_names, out_names, out_avals = [], [], []
    for alloc in nc.m.functions[0].allocations:
        if not isinstance(alloc, mybir_.MemoryLocationSet):
            continue
        name = alloc.memorylocations[0].name
        if alloc.kind == "ExternalInput":
            if name != partition_name:
                in_names.append(name)
        elif alloc.kind == "ExternalOutput":
            out_names.append(name)
            out_avals.append(
                jax.core.ShapedArray(
                    tuple(alloc.tensor_shape), mybir_.dt.np(alloc.dtype)
                )
            )
    n_params = len(in_names)
    all_names = in_names + out_names
    if partition_name is not None:
        all_names = all_names + [partition_name]

    def _body(*args):
        operands = list(args)
        if partition_name is not None:
            operands.append(partition_id_tensor())
        outs = _bass_exec_p.bind(
            *operands,
            out_avals=tuple(out_avals),
            in_names=tuple(all_names),
            out_names=tuple(out_names),
            lowering_input_output_aliases=(),
            sim_require_finite=True,
            sim_require_nnan=True,
            nc=nc,
        )
        return tuple(outs)

    devices = jax.devices()[:NCORES]
    mesh = Mesh(np.asarray(devices), ("core",))
    nin = n_params + len(out_names)
    sharded = jax.jit(
        shard_map(
            _body,
            mesh=mesh,
            in_specs=(PartitionSpec("core"),) * nin,
            out_specs=(PartitionSpec("core"),) * len(out_names),
            check_rep=False,
        ),
        keep_unused=True,
    )
    meta = (in_names, out_names, out_avals, n_params)
    _RUNNERS[nreps] = (sharded, meta)
    return _RUNNERS[nreps]


def run_repeated(inputs, nreps: int, iters: int = 6):
    """Run the nreps-unrolled NEFF; returns (out_core0, min_wall_seconds)."""
    import time
    import jax

    sharded, meta = _make_runner(nreps)
    in_names, out_names, out_avals, n_params = meta
    in_maps = _split_in_maps(inputs)
    concat_in = [
        np.concatenate([np.asarray(in_maps[c][n]) for c in range(NCORES)], axis=0)
        for n in in_names
    ]
    concat_zeros = [
        np.zeros((NCORES * a.shape[0], *a.shape[1:]), a.dtype) for a in out_avals
    ]
    args = [jax.device_put(a) for a in concat_in + concat_zeros]
    for a in args:
        a.block_until_ready()
    outs = sharded(*args)  # warmup/compile
    for o in outs:
        o.block_until_ready()
    times = []
    for _ in range(iters):
        t0 = time.perf_counter()
        outs = sharded(*args)
        for o in outs:
            o.block_until_ready()
        times.append(time.perf_counter() - t0)
    out = np.asarray(outs[0])
    return out, min(times)


def measure_exec_ns(inputs, k1=4, k2=36):
    """Amortized per-execution device time in ns via two-point slope."""
    out1, t1 = run_repeated(inputs, k1)
    out2, t2 = run_repeated(inputs, k2)
    per_exec = (t2 - t1) / (k2 - k1)
    return per_exec * 1e9, out2

